# revision 22
# baseline (speedup 1.0000x reference)
"""Trainium2 Bass kernel for nn_MixtureConfounderPrior.

Reference math (B,T,D=16,64,1024; K,CD,CF=32,128,128):
  cm  = 0.9*code_momentum + 0.1*code_embed
  mix = softmax(silu(h@mw_w1 + mw_b1) @ mw_w2 + mw_b2)
  mu_pre[t,k,c]  = (h@mu_w1[:D])[t,c] + (cm@mu_w1[D:])[k,c] + mu_b1[c]
  mus  = clip(tanh(LN(mu_pre)*g+b @ mu_w2 + mu_b2), -3, 3)
  lv   = clip((h@lv_w[:D])[t,c] + (cm@lv_w[D:])[k,c] + lv_b[c], LV_MIN, LV_MAX)

Key transformations:
  * mu_pre is rank-structured: A[t,c] + C[k,c].  LayerNorm stats collapse to
      mean[t,k] = mA[t]+mC[k],  var[t,k] = vA[t]+vC[k]+(2/CF)*(Ahat@Chat^T)[t,k]
  * the (t*k, CF)@(CF, CF) GEMM collapses to
      mus[t,k,f] = tanh(rstd[t,k]*(U[t,f]+V[k,f]) + bbias[f])
    with U = Ahat@(g*W2) on device and V = Chat@(g*W2) precomputed on host.
    The k-broadcasts are built in PSUM: U replicated via a 4x-tiled rhs,
    V/C_lv added via ones-row rank-1 accumulate matmuls.
  * clip(tanh(x),-3,3) == tanh(x); tanh(rstd*P) fused on ACT via per-partition
    scale = rstd[:,k].
  * silu(x) = 0.5*x*(1+tanh(x/2)); the 0.5 folds into mw_w2 so ACT needs only
    the exp/tanh table set (+ one Sqrt for rstd, ordered first).
  * matmuls with free dim >= 256 run in float32r (1 cyc/row vs 4 for fp32,
    ~1e-4 rel err).  PE transposes stay fp32 (fp32r transpose is broken on HW).

Data parallel over batch: 8 cores x 2 batches (128 tokens each); weights and
code-derived constants replicated.  No collectives; host gathers the slices.
"""

import math
from contextlib import ExitStack

import numpy as np

import concourse.bass as bass
import concourse.mybir as mybir
import concourse.tile as tile
from concourse.bass_utils import run_bass_kernel_spmd
from concourse.tile import add_dep_helper

B, T, D = 16, 64, 1024
K, CD, CF = 32, 128, 128
MOM = 0.9
LN_EPS = 1e-5
LV_MIN, LV_MAX = math.log(0.1), math.log(2.0)
NCORES = 8
BPC = B // NCORES          # batches per core
TOK = BPC * T              # 128 tokens per core
DCH = D // 128             # 8 contraction chunks
KG = 4                     # codes per PSUM bank group
NG = K // KG               # 8 bank groups
F32 = mybir.dt.float32
F32R = mybir.dt.float32r
AX = mybir.AluOpType
AF = mybir.ActivationFunctionType


def _split_drain_waits(nc, max_waits=1):
    """walrus in this env rejects >1 sem wait per instruction and any sem
    wait on a Drain.  Hoist them onto NoOps placed just before."""
    for f in nc.m.functions:
        for bb in f.blocks:
            new_list = []
            for inst in bb.instructions:
                si = inst.sync_info
                if si is not None and si.on_wait:
                    keep = 0 if isinstance(inst, mybir.InstDrain) else max_waits
                    if len(si.on_wait) > keep:
                        waits = list(si.on_wait)
                        head = waits[: len(waits) - keep]
                        for i, w in enumerate(head):
                            new_list.append(
                                mybir.InstNoOp(
                                    name=f"{inst.name}-wsplit{i}",
                                    engine=inst.engine,
                                    sync_info=mybir.SyncInfo(
                                        on_wait=[w], on_update=[]
                                    ),
                                )
                            )
                        si.on_wait = waits[len(waits) - keep :]
                new_list.append(inst)
            bb.instructions[:] = new_list


# pack32 column layout (f32 cols): h | ident | wml | w1s | w2g4 | w2m | chT
# | ones32(row0) | vcs(row0)
H0, ID, WML, W1S, W2G, I4C, W2M, CHT, ON32, VCS, P32_LEN = (
    0, 1024, 1152, 3200, 5248, 5760, 6272, 6336, 6368, 6496, 6528)
# smalls16 (bf16, partition 0): vflat | clvflat | ones16
SM_V, SM_C, SM_1, SM_LEN = 0, K * CF, 2 * K * CF, 2 * K * CF + 128
BF16 = mybir.dt.bfloat16


def build_bass(has_b1, has_b2, has_bb, split_waits=True):
    nc = bass.Bass("TRN2", num_devices=NCORES)

    def din(name, shape, dt=F32R):
        return nc.dram_tensor(name, shape, dt, kind="ExternalInput")

    p32_d = din("pack32", (128, P32_LEN))
    sm_d = din("smalls16", (1, SM_LEN), BF16)
    b1_d = din("b1row", (1, 256), F32) if has_b1 else None
    b2_d = din("b2row", (1, K), F32) if has_b2 else None
    bb_d = din("bbrep", (128, CF), F32) if has_bb else None

    mixw_d = nc.dram_tensor("mixw", (TOK, K), F32, kind="ExternalOutput")
    mus_d = nc.dram_tensor("mus", (TOK, K, CF), F32, kind="ExternalOutput")
    lv_d = nc.dram_tensor("lv", (TOK, K, CF), F32, kind="ExternalOutput")

    with tile.TileContext(nc) as tc, ExitStack() as ctx:
        cons = ctx.enter_context(tc.tile_pool(name="cons", bufs=1))
        stg = ctx.enter_context(tc.tile_pool(name="stg", bufs=3))

        # ---- loads: 4 chunk tiles from one DRAM pack (2 queues) ----------
        p32a = p32_d.ap()
        c1 = cons.tile([128, WML - H0], F32R, tag="c1", name="c1")
        nc.gpsimd.dma_start(c1, p32a[:, H0:WML])                 # h + ident
        c2 = cons.tile([128, W1S - WML], F32R, tag="c2", name="c2")
        nc.scalar.dma_start(c2, p32a[:, WML:W1S])                # wml
        c3 = cons.tile([128, W2G - W1S], F32R, tag="c3", name="c3")
        nc.sync.dma_start(c3, p32a[:, W1S:W2G])                  # w1s
        c4 = cons.tile([128, P32_LEN - W2G], F32R, tag="c4", name="c4")
        nc.scalar.dma_start(c4, p32a[:, W2G:P32_LEN])
        sm = cons.tile([1, SM_LEN], BF16, tag="sm", name="sm")
        nc.sync.dma_start(sm, sm_d.ap())

        h_sb = c1[:, H0:ID].bitcast(F32)
        ident = c1[:, ID:WML].bitcast(F32)
        wml = c2.rearrange("p (a b) -> p a b", b=256)
        w1s = c3.rearrange("p (a b) -> p a b", b=256)
        w2g4 = c4[:, 0 : I4C - W2G]
        i4r = c4[:, I4C - W2G : W2M - W2G]
        w2m = c4[:, W2M - W2G : CHT - W2G].bitcast(F32).rearrange(
            "p (a b) -> p a b", b=K)
        chT = c4[:, CHT - W2G : ON32 - W2G].bitcast(F32)
        ones32 = c4[0:1, ON32 - W2G : VCS - W2G].bitcast(F32)
        vcs = c4[0:1, VCS - W2G : P32_LEN - W2G].bitcast(F32)
        if has_b1:
            b1w = cons.tile([TOK, 256], F32, tag="b1w", name="b1w")
            nc.sync.dma_start(
                b1w, bass.AP(tensor=b1_d, offset=0, ap=[[0, TOK], [1, 256]]))
        if has_b2:
            b2r = cons.tile([1, K], F32, tag="b2r", name="b2r")
            nc.sync.dma_start(b2r, b2_d.ap())
        if has_bb:
            bbr = cons.tile([128, CF], F32, tag="bbr", name="bbr")
            nc.sync.dma_start(bbr, bb_d.ap())

        vflat = sm[:, SM_V : SM_V + K * CF]
        clv = sm[:, SM_C : SM_C + K * CF]
        ones16 = sm[:, SM_1 : SM_1 + 128]

        # mix-path PSUM lives in banks 0-1 for the whole kernel; phase-1
        # pools above it are closed before phase-2 claims banks 2-7.
        mixp = ctx.enter_context(tc.tile_pool(name="mixp", bufs=2,
                                              space="PSUM"))
        with tc.tile_pool(name="pt", bufs=3, space="PSUM") as ptp, \
             tc.tile_pool(name="mm", bufs=1, space="PSUM") as mmp, \
             tc.tile_pool(name="pss", bufs=1, space="PSUM") as pss, \
             tc.tile_pool(name="wrm", bufs=1, space="PSUM") as wrm:

            # PE warm-up: ~3.5us of dummy matmuls while input DMAs stream,
            # so the HAM clock-gate opens to 2.4 GHz before real work.
            scr = cons.tile([128, 512], F32, tag="scr", name="scr")
            nc.vector.memset(scr, 0.0)
            p_w = wrm.tile([128, 512], F32, tag="wrm")
            for _ in range(8):
                nc.tensor.matmul(p_w, lhsT=scr[:, 0:128], rhs=scr,
                                 start=True, stop=True)

            # ---- X^T: 8 PE transposes of h (fp32) ------------------------
            xT = cons.tile([128, DCH, TOK], F32R)
            for kc in range(DCH):
                pt = ptp.tile([128, 128], F32, tag="pt")
                nc.tensor.transpose(pt, h_sb[:, kc * 128 : (kc + 1) * 128],
                                    ident)
                nc.vector.tensor_copy(xT[:, kc, :], pt)

            # ---- A_mu | A_lv in one N=256 fp32r group --------------------
            p_ml = mmp.tile([TOK, 256], F32, tag="mm")
            for kc in range(DCH):
                nc.tensor.matmul(p_ml, lhsT=xT[:, kc, :], rhs=wml[:, kc, :],
                                 start=(kc == 0), stop=(kc == DCH - 1))
            stats = cons.tile([TOK, 6], F32)
            nc.vector.bn_stats(stats, p_ml[:, 0:CF])
            mv = cons.tile([TOK, 2], F32)
            nc.vector.bn_aggr(mv, stats)
            vae = cons.tile([TOK, 1], F32)
            nc.vector.tensor_scalar(vae, mv[:, 1:2], LN_EPS, None, AX.add)
            ahat = cons.tile([TOK, CF], F32)
            nc.vector.tensor_scalar(ahat, p_ml[:, 0:CF], mv[:, 0:1], None,
                                    AX.subtract)
            alv = cons.tile([TOK, CF], F32)
            nc.vector.tensor_copy(alv, p_ml[:, CF:256])

            p_at = ptp.tile([128, 128], F32, tag="pt")
            nc.tensor.transpose(p_at, ahat, ident)
            aT = cons.tile([CF, TOK], F32R)
            nc.scalar.copy(aT, p_at)
            p_lt = ptp.tile([128, 128], F32, tag="pt")
            nc.tensor.transpose(p_lt, alv, ident)
            alvT = cons.tile([CF, TOK], F32R)
            nc.scalar.copy(alvT, p_lt)

            # ---- rstd: var = vA + vC + (2/CF)*Ahat@ChatT (fp32, N=32) ----
            p_s = pss.tile([TOK, K], F32, tag="ps")
            nc.tensor.matmul(p_s, lhsT=aT.bitcast(F32), rhs=chT,
                             start=True, stop=False)
            nc.tensor.matmul(p_s, lhsT=ones32, rhs=vcs,
                             start=False, stop=True)
            sd = cons.tile([TOK, K], F32)
            sd_i = nc.scalar.activation(sd, p_s, AF.Sqrt, bias=vae,
                                        scale=2.0 / CF)
            rstd = cons.tile([TOK, K], F32)
            nc.vector.reciprocal(rstd, sd)

        # ---- mix path (PSUM banks 0-1, overlaps phase 2) -----------------
        p_y1 = mixp.tile([TOK, 256], F32, tag="mx", name="p_y1")
        for kc in range(DCH):
            nc.tensor.matmul(p_y1, lhsT=xT[:, kc, :], rhs=w1s[:, kc, :],
                             start=(kc == 0), stop=(kc == DCH - 1))
        th = cons.tile([TOK, 256], F32)
        y1 = cons.tile([TOK, 256], F32)
        if has_b1:
            nc.vector.tensor_tensor(y1, p_y1, b1w, AX.add)
            th_i = nc.scalar.activation(th, y1, AF.Tanh, scale=0.5)
        else:
            nc.vector.tensor_copy(y1, p_y1)
            th_i = nc.scalar.activation(th, p_y1, AF.Tanh, scale=0.5)
        # keep ACT table order: Sqrt before first Tanh
        add_dep_helper(th_i.ins, sd_i.ins, sync=False,
                       reason="ACT table-set order (sqrt first)")
        tmp = cons.tile([TOK, 256], F32)
        nc.vector.tensor_tensor(tmp, y1, th, AX.mult)
        y1s = cons.tile([TOK, 256], F32)
        nc.vector.tensor_tensor(y1s, tmp, y1, AX.add)

        y1sT = cons.tile([128, 2, TOK], F32)
        for j in range(2):
            p_yt = mixp.tile([128, 128], F32, tag="mx", name=f"p_yt{j}")
            nc.tensor.transpose(p_yt, y1s[:, j * 128 : (j + 1) * 128], ident)
            nc.scalar.copy(y1sT[:, j, :], p_yt)

        p_z = mixp.tile([TOK, K], F32, tag="mx", name="p_z")
        nc.tensor.matmul(p_z, lhsT=y1sT[:, 0, :], rhs=w2m[:, 0, :],
                         start=True, stop=False)
        nc.tensor.matmul(p_z, lhsT=y1sT[:, 1, :], rhs=w2m[:, 1, :],
                         start=False, stop=not has_b2)
        if has_b2:
            nc.tensor.matmul(p_z, lhsT=ones32, rhs=b2r,
                             start=False, stop=True)
        mx = cons.tile([TOK, 1], F32)
        nc.vector.reduce_max(mx, p_z, axis=mybir.AxisListType.X)
        nmx = cons.tile([TOK, 1], F32)
        nc.vector.tensor_scalar(nmx, mx, -1.0, None, AX.mult)
        ez = cons.tile([TOK, K], F32)
        esum = cons.tile([TOK, 1], F32)
        nc.scalar.activation(ez, p_z, AF.Exp, bias=nmx, accum_out=esum)
        rsum = cons.tile([TOK, 1], F32)
        nc.vector.reciprocal(rsum, esum)
        mixw = cons.tile([TOK, K], F32)
        nc.vector.tensor_scalar(mixw, ez, rsum, None, AX.mult)
        nc.sync.dma_start(mixw_d.ap(), mixw)

        # ---- big outputs: 4 pairs x (2-bank mus tile + 2-bank lv tile) ---
        KG2 = 2 * KG
        with tc.tile_pool(name="pP", bufs=2, space="PSUM") as psP, \
             tc.tile_pool(name="pL", bufs=1, space="PSUM") as psL:
            for pr in range(4):
                k0 = pr * KG2
                Pt = psP.tile([TOK, KG2, CF], F32, tag="pp", name=f"pp{pr}")
                nc.tensor.matmul(Pt[:, 0:KG, :], lhsT=aT, rhs=w2g4,
                                 start=True, stop=False)
                nc.tensor.matmul(Pt[:, KG:KG2, :], lhsT=aT, rhs=w2g4,
                                 start=True, stop=False)
                s0 = SM_V + k0 * CF
                nc.tensor.matmul(Pt[:, 0:KG, :], lhsT=ones16,
                                 rhs=sm[:, s0 : s0 + KG * CF],
                                 start=False, stop=True)
                nc.tensor.matmul(Pt[:, KG:KG2, :], lhsT=ones16,
                                 rhs=sm[:, s0 + KG * CF : s0 + KG2 * CF],
                                 start=False, stop=True)
                st = stg.tile([TOK, KG2, CF], F32, tag="st")
                nc.vector.tensor_tensor(
                    st, Pt,
                    rstd[:, k0 : k0 + KG2, None].to_broadcast(
                        (TOK, KG2, CF)),
                    AX.mult)
                if has_bb:
                    nc.vector.tensor_tensor(
                        st, st, bbr[:, None, :].to_broadcast((TOK, KG2, CF)),
                        AX.add)
                mus_sb = stg.tile([TOK, KG2, CF], F32, tag="mu")
                nc.scalar.activation(mus_sb, st, AF.Tanh)
                nc.sync.dma_start(mus_d.ap()[:, k0 : k0 + KG2, :], mus_sb)

                Lt = psL.tile([TOK, KG2, CF], F32, tag="pl", name=f"pl{pr}")
                nc.tensor.matmul(Lt[:, 0:KG, :], lhsT=alvT, rhs=i4r,
                                 start=True, stop=False)
                nc.tensor.matmul(Lt[:, KG:KG2, :], lhsT=alvT, rhs=i4r,
                                 start=True, stop=False)
                c0 = SM_C + k0 * CF
                nc.tensor.matmul(Lt[:, 0:KG, :], lhsT=ones16,
                                 rhs=sm[:, c0 : c0 + KG * CF],
                                 start=False, stop=True)
                nc.tensor.matmul(Lt[:, KG:KG2, :], lhsT=ones16,
                                 rhs=sm[:, c0 + KG * CF : c0 + KG2 * CF],
                                 start=False, stop=True)
                lv_sb = stg.tile([TOK, KG2, CF], F32, tag="lv")
                nc.vector.tensor_scalar(lv_sb, Lt, LV_MAX, LV_MIN,
                                        AX.min, AX.max)
                nc.scalar.dma_start(lv_d.ap()[:, k0 : k0 + KG2, :], lv_sb)

    if split_waits:
        _split_drain_waits(nc)
    return nc


def prepare(inputs):
    """Host-side preprocessing -> (in_maps, flags). All heavy per-token work
    stays on device; only (K,CD)-sized code/weight constants are folded."""
    f64 = {k: np.asarray(v, np.float64) for k, v in inputs.items()}
    h = np.ascontiguousarray(np.asarray(inputs["h"], np.float32))

    cm = MOM * f64["code_momentum"] + (1.0 - MOM) * f64["code_embed"]
    Cmu = cm @ f64["mu_w1"][D:] + f64["mu_b1"]          # (K, CF)
    mC = Cmu.mean(-1, keepdims=True)
    Chat = Cmu - mC
    vC = (Chat**2).mean(-1)                              # (K,)
    W2g = f64["ln_g"][:, None] * f64["mu_w2"]            # (CF, CF)
    V = Chat @ W2g                                       # (K, CF)
    bbias = f64["ln_b"] @ f64["mu_w2"] + f64["mu_b2"]    # (CF,)
    Clv = cm @ f64["lv_w"][D:] + f64["lv_b"]             # (K, CF)

    import ml_dtypes
    c = lambda a: np.ascontiguousarray(np.asarray(a, np.float32))
    w1s = c(f64["mw_w1"].reshape(DCH, 128, 256).transpose(1, 0, 2))
    wmu = f64["mu_w1"][:D].reshape(DCH, 128, CF).transpose(1, 0, 2)
    wlv = f64["lv_w"][:D].reshape(DCH, 128, CF).transpose(1, 0, 2)
    wml = c(np.concatenate([wmu, wlv], axis=2))          # (128, DCH, 256)
    w2m = c((0.5 * f64["mw_w2"]).reshape(2, 128, K).transpose(1, 0, 2))
    w2g4 = c(np.tile(W2g, (1, KG)))
    chT = c(Chat.T)
    pack32 = np.zeros((128, P32_LEN), np.float32)
    # h filled per core below
    pack32[:, ID:WML] = np.eye(128, dtype=np.float32)
    pack32[:, WML:W1S] = wml.reshape(128, -1)
    pack32[:, W1S:W2G] = w1s.reshape(128, -1)
    pack32[:, W2G:I4C] = w2g4
    pack32[:, I4C:W2M] = np.tile(np.eye(128), (1, KG))
    pack32[:, W2M:CHT] = w2m.reshape(128, -1)
    pack32[:, CHT:ON32] = chT
    pack32[0, ON32:VCS] = 1.0
    pack32[0, VCS:P32_LEN] = (CF / 2.0) * vC
    smalls = np.zeros((1, SM_LEN), ml_dtypes.bfloat16)
    smalls[0, SM_V : SM_V + K * CF] = V.reshape(-1).astype(ml_dtypes.bfloat16)
    smalls[0, SM_C : SM_C + K * CF] = Clv.reshape(-1).astype(
        ml_dtypes.bfloat16)
    smalls[0, SM_1 : SM_1 + 128] = 1.0

    has_b1 = bool(np.any(f64["mw_b1"]))
    has_b2 = bool(np.any(f64["mw_b2"]))
    has_bb = bool(np.any(bbias))

    common = dict(smalls16=smalls)
    if has_b1:
        common["b1row"] = c(f64["mw_b1"].reshape(1, 256))
    if has_b2:
        common["b2row"] = c(f64["mw_b2"].reshape(1, K))
    if has_bb:
        common["bbrep"] = c(np.tile(bbias.reshape(1, CF), (128, 1)))

    in_maps = []
    for i in range(NCORES):
        m = dict(common)
        p = pack32.copy()
        p[:, H0:ID] = h[i * BPC : (i + 1) * BPC].reshape(TOK, D)
        m["pack32"] = p
        in_maps.append(m)
    return in_maps, (has_b1, has_b2, has_bb)


_CACHE = {}


def run(inputs, **spmd_kwargs):
    in_maps, flags = prepare(inputs)
    if flags not in _CACHE:
        _CACHE[flags] = build_bass(*flags)
    nc = _CACHE[flags]
    res = run_bass_kernel_spmd(nc, in_maps, core_ids=list(range(NCORES)),
                               **spmd_kwargs)
    mix = np.empty((B, T, K), np.float32)
    mus = np.empty((B, T, K, CF), np.float32)
    lv = np.empty((B, T, K, CF), np.float32)
    for i, r in enumerate(res.results):
        sl = slice(i * BPC, (i + 1) * BPC)
        mix[sl] = r["mixw"].reshape(BPC, T, K)
        mus[sl] = r["mus"].reshape(BPC, T, K, CF)
        lv[sl] = r["lv"].reshape(BPC, T, K, CF)
    return (mix, mus, lv), res


def kernel(**inputs):
    out, _ = run(inputs)
    return out


# revision 23
# speedup vs baseline: 1.1456x; 1.1456x over previous
"""Trainium2 Bass kernel for nn_MixtureConfounderPrior.

Reference math (B,T,D=16,64,1024; K,CD,CF=32,128,128):
  cm  = 0.9*code_momentum + 0.1*code_embed
  mix = softmax(silu(h@mw_w1 + mw_b1) @ mw_w2 + mw_b2)
  mu_pre[t,k,c]  = (h@mu_w1[:D])[t,c] + (cm@mu_w1[D:])[k,c] + mu_b1[c]
  mus  = clip(tanh(LN(mu_pre)*g+b @ mu_w2 + mu_b2), -3, 3)
  lv   = clip((h@lv_w[:D])[t,c] + (cm@lv_w[D:])[k,c] + lv_b[c], LV_MIN, LV_MAX)

Key transformations:
  * mu_pre is rank-structured: A[t,c] + C[k,c].  LayerNorm stats collapse to
      mean[t,k] = mA[t]+mC[k],  var[t,k] = vA[t]+vC[k]+(2/CF)*(Ahat@Chat^T)[t,k]
  * the (t*k, CF)@(CF, CF) GEMM collapses to
      mus[t,k,f] = tanh(rstd[t,k]*(U[t,f]+V[k,f]) + bbias[f])
    with U = Ahat@(g*W2) on device and V = Chat@(g*W2) precomputed on host.
    The k-broadcasts are built in PSUM: U replicated via a 4x-tiled rhs,
    V/C_lv added via ones-row rank-1 accumulate matmuls.
  * clip(tanh(x),-3,3) == tanh(x); tanh(rstd*P) fused on ACT via per-partition
    scale = rstd[:,k].
  * silu(x) = 0.5*x*(1+tanh(x/2)); the 0.5 folds into mw_w2 so ACT needs only
    the exp/tanh table set (+ one Sqrt for rstd, ordered first).
  * matmuls with free dim >= 256 run in float32r (1 cyc/row vs 4 for fp32,
    ~1e-4 rel err).  PE transposes stay fp32 (fp32r transpose is broken on HW).

Data parallel over batch: 8 cores x 2 batches (128 tokens each); weights and
code-derived constants replicated.  No collectives; host gathers the slices.
"""

import math
from contextlib import ExitStack

import numpy as np

import concourse.bass as bass
import concourse.mybir as mybir
import concourse.tile as tile
from concourse.bass_utils import run_bass_kernel_spmd
from concourse.tile import add_dep_helper

B, T, D = 16, 64, 1024
K, CD, CF = 32, 128, 128
MOM = 0.9
LN_EPS = 1e-5
LV_MIN, LV_MAX = math.log(0.1), math.log(2.0)
NCORES = 8
BPC = B // NCORES          # batches per core
TOK = BPC * T              # 128 tokens per core
DCH = D // 128             # 8 contraction chunks
KG = 4                     # codes per PSUM bank group
NG = K // KG               # 8 bank groups
F32 = mybir.dt.float32
F32R = mybir.dt.float32r
AX = mybir.AluOpType
AF = mybir.ActivationFunctionType


def _split_drain_waits(nc, max_waits=1):
    """walrus in this env rejects >1 sem wait per instruction and any sem
    wait on a Drain.  Hoist them onto NoOps placed just before."""
    for f in nc.m.functions:
        for bb in f.blocks:
            new_list = []
            for inst in bb.instructions:
                si = inst.sync_info
                if si is not None and si.on_wait:
                    keep = 0 if isinstance(inst, mybir.InstDrain) else max_waits
                    if len(si.on_wait) > keep:
                        waits = list(si.on_wait)
                        head = waits[: len(waits) - keep]
                        for i, w in enumerate(head):
                            new_list.append(
                                mybir.InstNoOp(
                                    name=f"{inst.name}-wsplit{i}",
                                    engine=inst.engine,
                                    sync_info=mybir.SyncInfo(
                                        on_wait=[w], on_update=[]
                                    ),
                                )
                            )
                        si.on_wait = waits[len(waits) - keep :]
                new_list.append(inst)
            bb.instructions[:] = new_list


# pack32 column layout (f32 cols): h | ident | wml | w1s | w2g4 | w2m | chT
# | ones32(row0) | vcs(row0)
H0, ID, WML, W1S, W2G, I4C, W2M, CHT, ON32, VCS, P32_LEN = (
    0, 1024, 1152, 3200, 5248, 5760, 6272, 6336, 6368, 6496, 6528)
# smalls16 (bf16, partition 0): vflat | clvflat | ones16
SM_V, SM_C, SM_1, SM_LEN = 0, K * CF, 2 * K * CF, 2 * K * CF + 128
BF16 = mybir.dt.bfloat16


def build_bass(has_b1, has_b2, has_bb, split_waits=True):
    nc = bass.Bass("TRN2", num_devices=NCORES)

    def din(name, shape, dt=F32R):
        return nc.dram_tensor(name, shape, dt, kind="ExternalInput")

    p32_d = din("pack32", (128, P32_LEN))
    sm_d = din("smalls16", (1, SM_LEN), BF16)
    b1_d = din("b1row", (1, 256), F32) if has_b1 else None
    b2_d = din("b2row", (1, K), F32) if has_b2 else None
    bb_d = din("bbrep", (128, CF), F32) if has_bb else None

    mixw_d = nc.dram_tensor("mixw", (TOK, K), F32, kind="ExternalOutput")
    mus_d = nc.dram_tensor("mus", (TOK, K, CF), F32, kind="ExternalOutput")
    lv_d = nc.dram_tensor("lv", (TOK, K, CF), F32, kind="ExternalOutput")

    with tile.TileContext(nc) as tc, ExitStack() as ctx:
        cons = ctx.enter_context(tc.tile_pool(name="cons", bufs=1))
        stg = ctx.enter_context(tc.tile_pool(name="stg", bufs=3))

        # ---- loads: 4 chunk tiles from one DRAM pack (2 queues) ----------
        p32a = p32_d.ap()
        c1 = cons.tile([128, WML - H0], F32R, tag="c1", name="c1")
        nc.sync.dma_start(c1, p32a[:, H0:WML])                   # h + ident
        c2 = cons.tile([128, W1S - WML], F32R, tag="c2", name="c2")
        nc.scalar.dma_start(c2, p32a[:, WML:W1S])                # wml
        c3 = cons.tile([128, W2G - W1S], F32R, tag="c3", name="c3")
        nc.sync.dma_start(c3, p32a[:, W1S:W2G])                  # w1s (after c1)
        c4 = cons.tile([128, P32_LEN - W2G], F32R, tag="c4", name="c4")
        nc.scalar.dma_start(c4, p32a[:, W2G:P32_LEN])
        sm = cons.tile([1, SM_LEN], BF16, tag="sm", name="sm")
        nc.sync.dma_start(sm, sm_d.ap())

        h_sb = c1[:, H0:ID].bitcast(F32)
        ident = c1[:, ID:WML].bitcast(F32)
        wml = c2.rearrange("p (a b) -> p a b", b=256)
        w1s = c3.rearrange("p (a b) -> p a b", b=256)
        w2g4 = c4[:, 0 : I4C - W2G]
        i4r = c4[:, I4C - W2G : W2M - W2G]
        w2m = c4[:, W2M - W2G : CHT - W2G].bitcast(F32).rearrange(
            "p (a b) -> p a b", b=K)
        chT = c4[:, CHT - W2G : ON32 - W2G].bitcast(F32)
        ones32 = c4[0:1, ON32 - W2G : VCS - W2G].bitcast(F32)
        vcs = c4[0:1, VCS - W2G : P32_LEN - W2G].bitcast(F32)
        if has_b1:
            b1w = cons.tile([TOK, 256], F32, tag="b1w", name="b1w")
            nc.sync.dma_start(
                b1w, bass.AP(tensor=b1_d, offset=0, ap=[[0, TOK], [1, 256]]))
        if has_b2:
            b2r = cons.tile([1, K], F32, tag="b2r", name="b2r")
            nc.sync.dma_start(b2r, b2_d.ap())
        if has_bb:
            bbr = cons.tile([128, CF], F32, tag="bbr", name="bbr")
            nc.sync.dma_start(bbr, bb_d.ap())

        vflat = sm[:, SM_V : SM_V + K * CF]
        clv = sm[:, SM_C : SM_C + K * CF]
        ones16 = sm[:, SM_1 : SM_1 + 128]

        # mix-path PSUM lives in banks 0-1 for the whole kernel; phase-1
        # pools above it are closed before phase-2 claims banks 2-7.
        mixp = ctx.enter_context(tc.tile_pool(name="mixp", bufs=2,
                                              space="PSUM"))
        with tc.tile_pool(name="pt", bufs=3, space="PSUM") as ptp, \
             tc.tile_pool(name="mm", bufs=1, space="PSUM") as mmp, \
             tc.tile_pool(name="pss", bufs=1, space="PSUM") as pss, \
             tc.tile_pool(name="wrm", bufs=1, space="PSUM") as wrm:

            # PE warm-up: ~3.5us of dummy matmuls while input DMAs stream,
            # so the HAM clock-gate opens to 2.4 GHz before real work.
            scr = cons.tile([128, 512], BF16, tag="scr", name="scr")
            nc.vector.memset(scr, 0.0)
            p_w = wrm.tile([128, 512], F32, tag="wrm")
            for _ in range(10):
                nc.tensor.matmul(p_w, lhsT=scr[:, 0:128], rhs=scr,
                                 start=True, stop=True)

            # ---- X^T: 8 PE transposes of h (fp32) ------------------------
            xT = cons.tile([128, DCH, TOK], F32R)
            for kc in range(DCH):
                pt = ptp.tile([128, 128], F32, tag="pt")
                nc.tensor.transpose(pt, h_sb[:, kc * 128 : (kc + 1) * 128],
                                    ident)
                nc.vector.tensor_copy(xT[:, kc, :], pt)

            # ---- A_mu | A_lv in one N=256 fp32r group --------------------
            p_ml = mmp.tile([TOK, 256], F32, tag="mm")
            for kc in range(DCH):
                nc.tensor.matmul(p_ml, lhsT=xT[:, kc, :], rhs=wml[:, kc, :],
                                 start=(kc == 0), stop=(kc == DCH - 1))
            stats = cons.tile([TOK, 6], F32)
            nc.vector.bn_stats(stats, p_ml[:, 0:CF])
            mv = cons.tile([TOK, 2], F32)
            nc.vector.bn_aggr(mv, stats)
            vae = cons.tile([TOK, 1], F32)
            nc.vector.tensor_scalar(vae, mv[:, 1:2], LN_EPS, None, AX.add)
            ahat = cons.tile([TOK, CF], F32)
            nc.vector.tensor_scalar(ahat, p_ml[:, 0:CF], mv[:, 0:1], None,
                                    AX.subtract)
            alv = cons.tile([TOK, CF], F32)
            nc.vector.tensor_copy(alv, p_ml[:, CF:256])

            p_at = ptp.tile([128, 128], F32, tag="pt")
            nc.tensor.transpose(p_at, ahat, ident)
            aT = cons.tile([CF, TOK], F32R)
            nc.scalar.copy(aT, p_at)
            p_lt = ptp.tile([128, 128], F32, tag="pt")
            nc.tensor.transpose(p_lt, alv, ident)
            alvT = cons.tile([CF, TOK], F32R)
            nc.scalar.copy(alvT, p_lt)

            # ---- rstd: var = vA + vC + (2/CF)*Ahat@ChatT (fp32, N=32) ----
            p_s = pss.tile([TOK, K], F32, tag="ps")
            nc.tensor.matmul(p_s, lhsT=aT.bitcast(F32), rhs=chT,
                             start=True, stop=False)
            nc.tensor.matmul(p_s, lhsT=ones32, rhs=vcs,
                             start=False, stop=True)
            sd = cons.tile([TOK, K], F32)
            sd_i = nc.scalar.activation(sd, p_s, AF.Sqrt, bias=vae,
                                        scale=2.0 / CF)
            rstd = cons.tile([TOK, K], F32)
            nc.vector.reciprocal(rstd, sd)

        # ---- mix path (PSUM banks 0-1, overlaps phase 2) -----------------
        p_y1 = mixp.tile([TOK, 256], F32, tag="mx", name="p_y1")
        for kc in range(DCH):
            nc.tensor.matmul(p_y1, lhsT=xT[:, kc, :], rhs=w1s[:, kc, :],
                             start=(kc == 0), stop=(kc == DCH - 1))
        th = cons.tile([TOK, 256], F32)
        y1 = cons.tile([TOK, 256], F32)
        if has_b1:
            nc.vector.tensor_tensor(y1, p_y1, b1w, AX.add)
            th_i = nc.scalar.activation(th, y1, AF.Tanh, scale=0.5)
        else:
            nc.vector.tensor_copy(y1, p_y1)
            th_i = nc.scalar.activation(th, p_y1, AF.Tanh, scale=0.5)
        # keep ACT table order: Sqrt before first Tanh
        add_dep_helper(th_i.ins, sd_i.ins, sync=False,
                       reason="ACT table-set order (sqrt first)")
        tmp = cons.tile([TOK, 256], F32)
        nc.vector.tensor_tensor(tmp, y1, th, AX.mult)
        y1s = cons.tile([TOK, 256], F32)
        nc.vector.tensor_tensor(y1s, tmp, y1, AX.add)

        y1sT = cons.tile([128, 2, TOK], F32)
        for j in range(2):
            p_yt = mixp.tile([128, 128], F32, tag="mx", name=f"p_yt{j}")
            nc.tensor.transpose(p_yt, y1s[:, j * 128 : (j + 1) * 128], ident)
            nc.scalar.copy(y1sT[:, j, :], p_yt)

        p_z = mixp.tile([TOK, K], F32, tag="mx", name="p_z")
        nc.tensor.matmul(p_z, lhsT=y1sT[:, 0, :], rhs=w2m[:, 0, :],
                         start=True, stop=False)
        nc.tensor.matmul(p_z, lhsT=y1sT[:, 1, :], rhs=w2m[:, 1, :],
                         start=False, stop=not has_b2)
        if has_b2:
            nc.tensor.matmul(p_z, lhsT=ones32, rhs=b2r,
                             start=False, stop=True)
        mx = cons.tile([TOK, 1], F32)
        nc.vector.reduce_max(mx, p_z, axis=mybir.AxisListType.X)
        nmx = cons.tile([TOK, 1], F32)
        nc.vector.tensor_scalar(nmx, mx, -1.0, None, AX.mult)
        ez = cons.tile([TOK, K], F32)
        esum = cons.tile([TOK, 1], F32)
        nc.scalar.activation(ez, p_z, AF.Exp, bias=nmx, accum_out=esum)
        rsum = cons.tile([TOK, 1], F32)
        nc.vector.reciprocal(rsum, esum)
        mixw = cons.tile([TOK, K], F32)
        nc.vector.tensor_scalar(mixw, ez, rsum, None, AX.mult)
        nc.sync.dma_start(mixw_d.ap(), mixw)

        # ---- big outputs: 4 pairs x (2-bank mus tile + 2-bank lv tile) ---
        KG2 = 2 * KG
        with tc.tile_pool(name="pP", bufs=2, space="PSUM") as psP, \
             tc.tile_pool(name="pL", bufs=1, space="PSUM") as psL:
            for pr in range(4):
                k0 = pr * KG2
                Pt = psP.tile([TOK, KG2, CF], F32, tag="pp", name=f"pp{pr}")
                nc.tensor.matmul(Pt[:, 0:KG, :], lhsT=aT, rhs=w2g4,
                                 start=True, stop=False)
                nc.tensor.matmul(Pt[:, KG:KG2, :], lhsT=aT, rhs=w2g4,
                                 start=True, stop=False)
                s0 = SM_V + k0 * CF
                nc.tensor.matmul(Pt[:, 0:KG, :], lhsT=ones16,
                                 rhs=sm[:, s0 : s0 + KG * CF],
                                 start=False, stop=True)
                nc.tensor.matmul(Pt[:, KG:KG2, :], lhsT=ones16,
                                 rhs=sm[:, s0 + KG * CF : s0 + KG2 * CF],
                                 start=False, stop=True)
                st = stg.tile([TOK, KG2, CF], F32, tag="st")
                nc.vector.tensor_tensor(
                    st, Pt,
                    rstd[:, k0 : k0 + KG2, None].to_broadcast(
                        (TOK, KG2, CF)),
                    AX.mult)
                if has_bb:
                    nc.vector.tensor_tensor(
                        st, st, bbr[:, None, :].to_broadcast((TOK, KG2, CF)),
                        AX.add)
                mus_sb = stg.tile([TOK, KG2, CF], F32, tag="mu")
                nc.scalar.activation(mus_sb, st, AF.Tanh)
                nc.sync.dma_start(mus_d.ap()[:, k0 : k0 + KG2, :], mus_sb)

                Lt = psL.tile([TOK, KG2, CF], F32, tag="pl", name=f"pl{pr}")
                nc.tensor.matmul(Lt[:, 0:KG, :], lhsT=alvT, rhs=i4r,
                                 start=True, stop=False)
                nc.tensor.matmul(Lt[:, KG:KG2, :], lhsT=alvT, rhs=i4r,
                                 start=True, stop=False)
                c0 = SM_C + k0 * CF
                nc.tensor.matmul(Lt[:, 0:KG, :], lhsT=ones16,
                                 rhs=sm[:, c0 : c0 + KG * CF],
                                 start=False, stop=True)
                nc.tensor.matmul(Lt[:, KG:KG2, :], lhsT=ones16,
                                 rhs=sm[:, c0 + KG * CF : c0 + KG2 * CF],
                                 start=False, stop=True)
                lv_sb = stg.tile([TOK, KG2, CF], F32, tag="lv")
                nc.vector.tensor_scalar(lv_sb, Lt, LV_MAX, LV_MIN,
                                        AX.min, AX.max)
                nc.scalar.dma_start(lv_d.ap()[:, k0 : k0 + KG2, :], lv_sb)

    if split_waits:
        _split_drain_waits(nc)
    return nc


def prepare(inputs):
    """Host-side preprocessing -> (in_maps, flags). All heavy per-token work
    stays on device; only (K,CD)-sized code/weight constants are folded."""
    f64 = {k: np.asarray(v, np.float64) for k, v in inputs.items()}
    h = np.ascontiguousarray(np.asarray(inputs["h"], np.float32))

    cm = MOM * f64["code_momentum"] + (1.0 - MOM) * f64["code_embed"]
    Cmu = cm @ f64["mu_w1"][D:] + f64["mu_b1"]          # (K, CF)
    mC = Cmu.mean(-1, keepdims=True)
    Chat = Cmu - mC
    vC = (Chat**2).mean(-1)                              # (K,)
    W2g = f64["ln_g"][:, None] * f64["mu_w2"]            # (CF, CF)
    V = Chat @ W2g                                       # (K, CF)
    bbias = f64["ln_b"] @ f64["mu_w2"] + f64["mu_b2"]    # (CF,)
    Clv = cm @ f64["lv_w"][D:] + f64["lv_b"]             # (K, CF)

    import ml_dtypes
    c = lambda a: np.ascontiguousarray(np.asarray(a, np.float32))
    w1s = c(f64["mw_w1"].reshape(DCH, 128, 256).transpose(1, 0, 2))
    wmu = f64["mu_w1"][:D].reshape(DCH, 128, CF).transpose(1, 0, 2)
    wlv = f64["lv_w"][:D].reshape(DCH, 128, CF).transpose(1, 0, 2)
    wml = c(np.concatenate([wmu, wlv], axis=2))          # (128, DCH, 256)
    w2m = c((0.5 * f64["mw_w2"]).reshape(2, 128, K).transpose(1, 0, 2))
    w2g4 = c(np.tile(W2g, (1, KG)))
    chT = c(Chat.T)
    pack32 = np.zeros((128, P32_LEN), np.float32)
    # h filled per core below
    pack32[:, ID:WML] = np.eye(128, dtype=np.float32)
    pack32[:, WML:W1S] = wml.reshape(128, -1)
    pack32[:, W1S:W2G] = w1s.reshape(128, -1)
    pack32[:, W2G:I4C] = w2g4
    pack32[:, I4C:W2M] = np.tile(np.eye(128), (1, KG))
    pack32[:, W2M:CHT] = w2m.reshape(128, -1)
    pack32[:, CHT:ON32] = chT
    pack32[0, ON32:VCS] = 1.0
    pack32[0, VCS:P32_LEN] = (CF / 2.0) * vC
    smalls = np.zeros((1, SM_LEN), ml_dtypes.bfloat16)
    smalls[0, SM_V : SM_V + K * CF] = V.reshape(-1).astype(ml_dtypes.bfloat16)
    smalls[0, SM_C : SM_C + K * CF] = Clv.reshape(-1).astype(
        ml_dtypes.bfloat16)
    smalls[0, SM_1 : SM_1 + 128] = 1.0

    has_b1 = bool(np.any(f64["mw_b1"]))
    has_b2 = bool(np.any(f64["mw_b2"]))
    has_bb = bool(np.any(bbias))

    common = dict(smalls16=smalls)
    if has_b1:
        common["b1row"] = c(f64["mw_b1"].reshape(1, 256))
    if has_b2:
        common["b2row"] = c(f64["mw_b2"].reshape(1, K))
    if has_bb:
        common["bbrep"] = c(np.tile(bbias.reshape(1, CF), (128, 1)))

    in_maps = []
    for i in range(NCORES):
        m = dict(common)
        p = pack32.copy()
        p[:, H0:ID] = h[i * BPC : (i + 1) * BPC].reshape(TOK, D)
        m["pack32"] = p
        in_maps.append(m)
    return in_maps, (has_b1, has_b2, has_bb)


_CACHE = {}


def run(inputs, **spmd_kwargs):
    in_maps, flags = prepare(inputs)
    if flags not in _CACHE:
        _CACHE[flags] = build_bass(*flags)
    nc = _CACHE[flags]
    res = run_bass_kernel_spmd(nc, in_maps, core_ids=list(range(NCORES)),
                               **spmd_kwargs)
    mix = np.empty((B, T, K), np.float32)
    mus = np.empty((B, T, K, CF), np.float32)
    lv = np.empty((B, T, K, CF), np.float32)
    for i, r in enumerate(res.results):
        sl = slice(i * BPC, (i + 1) * BPC)
        mix[sl] = r["mixw"].reshape(BPC, T, K)
        mus[sl] = r["mus"].reshape(BPC, T, K, CF)
        lv[sl] = r["lv"].reshape(BPC, T, K, CF)
    return (mix, mus, lv), res


def kernel(**inputs):
    out, _ = run(inputs)
    return out
